# revision 1
# baseline (speedup 1.0000x reference)
"""Sliding-window causal GQA attention (RoPE) for Trainium2, 8-core SPMD.

Problem: x:(4,2048,2048), Wq:(2048,2048), Wk/Wv:(512,2048), Wo:(2048,2048)
  q = rope(x @ Wq.T) 16 heads, k/v = (x @ Wk.T / x @ Wv.T) 4 kv heads (GQA x4),
  causal sliding-window attention (W=1024), out = z @ Wo.T.

Sharding: 8 cores = 4 batches x 2 head-groups (8 q-heads / 2 kv-heads each).
Each core computes a partial output (its head-group's Wo contribution) for its
batch; host sums the two partials per batch.

Per-core kernel (projections/scores in f32r; exp'd probabilities bf16):
  - layout: qT/kT as (head_dim, L) ["transposed"], v as (L, head_dim)
  - scores computed transposed S.T (keys on partitions, queries free) so P.T
    feeds the PV matmul directly with no on-chip transposes.
  - softmax denominator: ones[128,128] stationary matmul accumulates the
    per-query sum broadcast across all 128 partitions directly in PSUM
    (no separate M=1 sum + K=1 broadcast matmuls).
  - no max-subtraction in softmax: logits are O(1) here, exp is safe.
  - sliding window at 128-block granularity: query-super of 256 x up to 10
    key-blocks; boundary blocks masked via precomputed 0/1 tiles.
  - lag-2 software pipeline: the denominator/PV of super t are issued two
    score-slots later, so the PE never waits on exp/mask/RoPE latency.
  - inputs are host-prepacked so each DMA moves long contiguous runs per
    partition, keeping DMA packet counts low.
"""

import math
import numpy as np

H = 16
D = 4
WINDOW = 1024
THETA = 10000.0
N, L, E = 4, 2048, 2048
P = 128
DH = E // H          # 128 head dim
NH = H // 2          # 8 q heads per core
NKV = 2              # kv heads per core
NB = L // P          # 16 key blocks
NKT = E // P         # 16 contraction tiles
SCALE = 1.0 / math.sqrt(DH)

_NC = None


def _kbs_for_super(t):
    """Key blocks overlapping the window of query super t (256 queries)."""
    return list(range(max(0, 2 * t - 8), 2 * t + 2))


def build_nc():
    from contextlib import ExitStack
    from concourse import bacc, tile, mybir

    F32 = mybir.dt.float32
    F32R = mybir.dt.float32r
    BF16 = mybir.dt.bfloat16
    EXP = mybir.ActivationFunctionType.Exp

    SHUF_SWAP = [i ^ 1 for i in range(32)]

    nc = bacc.Bacc("TRN2", target_bir_lowering=False, debug=False)
    # prepacked inputs (see _pack_core_inputs for layouts)
    xq = nc.dram_tensor("xq", [4 * P, NKT * 512], F32R, kind="ExternalInput").ap()
    wqp = nc.dram_tensor("wqp", [NH * P, NKT * DH], F32R, kind="ExternalInput").ap()
    wkv = nc.dram_tensor("wkv", [P, NKT * 512], F32R, kind="ExternalInput").ap()
    woT = nc.dram_tensor("woT", [NH * DH, E], BF16, kind="ExternalInput").ap()
    cosT = nc.dram_tensor("cosT", [P, L], F32, kind="ExternalInput").ap()
    sinT = nc.dram_tensor("sinT", [P, L], F32, kind="ExternalInput").ap()
    masks = nc.dram_tensor("masks", [4 * P, 256], BF16, kind="ExternalInput").ap()
    out = nc.dram_tensor("out", [L, E], F32, kind="ExternalOutput").ap()
    zspill = nc.dram_tensor("zspill", [NH * P, L], BF16).ap()

    with tile.TileContext(nc) as tc, ExitStack() as stk:
        resid = stk.enter_context(tc.tile_pool(name="resid", bufs=1))
        kT = [resid.tile([P, L], F32R, tag=f"kT{i}", name=f"kT{i}") for i in range(NKV)]
        kvw = resid.tile([P, NKT * 512], F32R, tag="kvw")
        vt = [[resid.tile([P, P], BF16, tag=f"v{i}_{b}", name=f"v{i}_{b}") for b in range(NB)]
              for i in range(NKV)]

        z3 = [resid.tile([P, 512], BF16, tag=f"z3_{h}", name=f"z3_{h}")
              for h in range(NH)]
        const = stk.enter_context(tc.tile_pool(name="const", bufs=1))
        # mask kinds: 0=diagA (k<=q), 1=diagB (k<=q-128),
        #             2=farA (k>=q+1), 3=farB (k>=q-127)
        mk = [const.tile([P, 256], BF16, tag=f"mk{i}", name=f"mk{i}") for i in range(4)]
        ones_f = const.tile([P, P], F32, tag="ones_f")
        ones = const.tile([P, P], BF16, tag="ones")

        def rope_evict(dest, psum, cos_sl, sin_sl, tmp_pool, n):
            # dest = psum * cos + pairswap(psum) * sin   (sin pre-signed)
            tmp = tmp_pool.tile([P, 512], F32, tag="ropetmp", name="ropetmp")
            nc.vector.stream_shuffle(tmp[:, :n], psum, SHUF_SWAP)
            nc.vector.tensor_mul(tmp[:, :n], tmp[:, :n], sin_sl)
            nc.vector.tensor_mul(dest, psum, cos_sl)
            nc.vector.tensor_add(dest, dest, tmp[:, :n])

        osb = stk.enter_context(tc.tile_pool(name="osb", bufs=3))
        pacc = stk.enter_context(tc.tile_pool(name="pacc", bufs=5, space="PSUM"))
        pzp = stk.enter_context(tc.tile_pool(name="pz", bufs=2, space="PSUM"))
        pbp = stk.enter_context(tc.tile_pool(name="pb", bufs=1, space="PSUM"))
        with tc.tile_pool(name="quarter", bufs=2) as qpool, \
             tc.tile_pool(name="wq", bufs=3) as wqpool, \
             tc.tile_pool(name="work", bufs=4) as work, \
             tc.tile_pool(name="qt", bufs=3) as qtpool, \
             tc.tile_pool(name="zev", bufs=3) as zevpool, \
             tc.tile_pool(name="rtmp", bufs=2) as rtmp:

            def load_quarter(qtr):
                xt = qpool.tile([P, NKT * 512], F32R, tag="xt")
                cos_q = qpool.tile([P, 512], F32, tag="cos")
                sin_q = qpool.tile([P, 512], F32, tag="sin")
                for dc in range(4):
                    nc.sync.dma_start(
                        out=xt[:, dc * 2048:(dc + 1) * 2048],
                        in_=xq[qtr * P:(qtr + 1) * P, dc * 2048:(dc + 1) * 2048])
                c0 = 512 * qtr
                nc.sync.dma_start(out=cos_q[:], in_=cosT[:, c0:c0 + 512])
                nc.sync.dma_start(out=sin_q[:], in_=sinT[:, c0:c0 + 512])
                return xt, cos_q, sin_q

            # startup: compute inputs first, then small tables
            for dc in range(4):
                nc.sync.dma_start(out=kvw[:, dc * 2048:(dc + 1) * 2048],
                                  in_=wkv[:, dc * 2048:(dc + 1) * 2048])
            cur = load_quarter(0)

            # rolling Wq prefetch, 3 heads deep (global head index)
            wq_q = []

            def wq_prefetch(g):
                if g >= 4 * NH:
                    return
                h = g % NH
                wqt = wqpool.tile([P, NKT * DH], F32R, tag="wqh", name="wqt")
                nc.sync.dma_start(out=wqt[:], in_=wqp[h * P:(h + 1) * P, :])
                wq_q.append(wqt)

            for i in range(4):
                nc.sync.dma_start(out=mk[i][:], in_=masks[i * P:(i + 1) * P, :])
            nc.vector.memset(ones_f[:], 1.0)
            nc.vector.tensor_copy(ones[:], ones_f[:])

            # lag-2 pipeline of attention tails
            pend = []

            def attn_tail():
                kv, kbs, pt, h, t = pend.pop(0)
                nkb = len(kbs)

                def qspan(kb):
                    # diagB (kb==2t+1) is all-masked for the first 128 queries
                    # of the super; farA (kb==2t-8) for the last 128. Skip the
                    # dead half in the denominator/PV matmuls (partial-width
                    # PSUM accumulation zero-fills untouched columns).
                    if kb == 2 * t + 1:
                        return 128, 256
                    if kb == 2 * t - 8:
                        return 0, 128
                    return 0, 256

                # denominator, broadcast across partitions (ones stationary)
                pb = pbp.tile([P, 256], F32, tag="pb")
                for i, kb in enumerate(kbs):
                    a, b = qspan(kb)
                    nc.tensor.matmul(
                        pb[:, a:b], ones[:],
                        pt[:, i * 256 + a:i * 256 + b],
                        start=(i == 0), stop=(i == nkb - 1))
                # PV
                pz = pzp.tile([P, 256], F32, tag="pz")
                for i, kb in enumerate(kbs):
                    a, b = qspan(kb)
                    nc.tensor.matmul(
                        pz[:, a:b], vt[kv][kb][:],
                        pt[:, i * 256 + a:i * 256 + b],
                        start=(i == 0), stop=(i == nkb - 1))
                rec = zevpool.tile([P, 256], F32, tag="rec")
                nc.vector.reciprocal_approx_fast(rec[:], pb[:])
                if t >= 6:
                    s3 = t - 6
                    nc.vector.tensor_mul(
                        z3[h][:, s3 * 256:(s3 + 1) * 256], pz[:], rec[:])
                else:
                    zev = zevpool.tile([P, 256], BF16, tag="zev")
                    nc.vector.tensor_mul(zev[:], pz[:], rec[:])
                    nc.sync.dma_start(
                        out=zspill[h * P:(h + 1) * P, t * 256:(t + 1) * 256],
                        in_=zev[:])

            for qtr in range(4):
                xt, cos_q, sin_q = cur

                def xtile(kt, a, b):
                    return xt[:, kt * 512 + a: kt * 512 + b]

                c0 = 512 * qtr

                # K projection (+RoPE) for both kv heads
                for kv in range(NKV):
                    pk = pacc.tile([P, 512], F32, tag="pacc")
                    for kt in range(NKT):
                        nc.tensor.matmul(
                            pk[:],
                            kvw[:, kt * 512 + kv * DH: kt * 512 + (kv + 1) * DH],
                            xtile(kt, 0, 512),
                            start=(kt == 0), stop=(kt == NKT - 1),
                        )
                    rope_evict(kT[kv][:, c0:c0 + 512], pk[:], cos_q[:], sin_q[:], rtmp, 512)

                if qtr == 0:
                    # deferred so these 3MB don't share DMA bandwidth with
                    # the critical kvw+xt startup burst
                    for g0 in range(3):
                        wq_prefetch(g0)

                # V projection (both kv heads at once, natural layout)
                for lb in range(4):
                    pv = pacc.tile([P, 512], F32, tag="pacc")
                    for kt in range(NKT):
                        nc.tensor.matmul(
                            pv[:, :NKV * DH],
                            xtile(kt, lb * P, (lb + 1) * P),
                            kvw[:, kt * 512 + 256: kt * 512 + 512],
                            start=(kt == 0), stop=(kt == NKT - 1),
                        )
                    for kv in range(NKV):
                        nc.scalar.copy(vt[kv][4 * qtr + lb][:], pv[:, kv * DH:(kv + 1) * DH])

                def q_proj(g):
                    """Q projection + RoPE for one head; returns the roped tile."""
                    wq = wq_q.pop(0)
                    wq_prefetch(g + 3)
                    pq = pacc.tile([P, 512], F32, tag="pacc")
                    for kt in range(NKT):
                        nc.tensor.matmul(
                            pq[:],
                            wq[:, kt * DH:(kt + 1) * DH],
                            xtile(kt, 0, 512),
                            start=(kt == 0), stop=(kt == NKT - 1),
                        )
                    qth = qtpool.tile([P, 512], F32R, tag="qt")
                    rope_evict(qth[:], pq[:], cos_q[:], sin_q[:], rtmp, 512)
                    return qth

                # Q projection + attention, head-major; Q runs one head ahead
                # so RoPE latency is always covered by PE work
                qnext = q_proj(qtr * NH)
                for h in range(NH):
                    kv = h // (NH // NKV)
                    qth = qnext
                    if h + 1 < NH:
                        qnext = q_proj(qtr * NH + h + 1)
                    if h == 1 and qtr < 3:
                        cur = load_quarter(qtr + 1)
                    for s in range(2):
                        t = 2 * qtr + s
                        qt = qth[:, s * 256:(s + 1) * 256]

                        # drain the oldest pending super (lag 2)
                        if len(pend) >= 2:
                            attn_tail()

                        kbs = _kbs_for_super(t)
                        nkb = len(kbs)
                        pt = work.tile([P, 2560], BF16, tag="pt")
                        # scores (transposed: keys on partitions) in chunks of 2 kb
                        for ci in range(0, nkb, 2):
                            cn = min(2, nkb - ci)
                            ps = pacc.tile([P, 512], F32, tag="pacc")
                            for i in range(cn):
                                kb = kbs[ci + i]
                                nc.tensor.matmul(
                                    ps[:, i * 256:(i + 1) * 256],
                                    kT[kv][:, kb * P:(kb + 1) * P],
                                    qt,
                                    start=True, stop=True,
                                )
                            nc.scalar.activation(
                                pt[:, ci * 256:(ci + cn) * 256],
                                ps[:, :cn * 256], EXP, scale=SCALE)
                        # window masks on boundary blocks (only the live
                        # query-halves of diagB/farA; the dead halves are
                        # skipped by the denominator/PV matmuls)
                        for i, kb in enumerate(kbs):
                            if kb == 2 * t:
                                kind, a, b = 0, 0, 256
                            elif kb == 2 * t + 1:
                                kind, a, b = 1, 128, 256
                            elif kb == 2 * t - 8:
                                kind, a, b = 2, 0, 128
                            elif kb == 2 * t - 7:
                                kind, a, b = 3, 0, 256
                            else:
                                continue
                            sl = pt[:, i * 256 + a:i * 256 + b]
                            nc.vector.tensor_mul(sl, sl, mk[kind][:, a:b])
                        pend.append((kv, kbs, pt, h, t))

            while pend:
                attn_tail()

        # Output projection: out[q,:] += sum_h zTn_h[:,q].T @ woT[h]
        # wo / z ship as bf16 and are converted on-chip (DVE is idle here);
        # the h-loop is outermost per ec-slice so the first matmuls only
        # need wo[0], overlapping the wo loads with compute.
        with tc.tile_pool(name="wo", bufs=1) as wopool, \
             tc.tile_pool(name="wostg", bufs=1) as wostg, \
             tc.tile_pool(name="zinb", bufs=4) as zinbpool, \
             tc.tile_pool(name="zin", bufs=2) as zinpool:
            # wo ships bf16; convert ec-major so the first output column
            # only waits for 512-wide pieces, not whole-head converts
            stg = [wostg.tile([P, E], BF16, tag=f"wostg{h}", name=f"stg{h}")
                   for h in range(NH)]
            wo = [wopool.tile([P, E], F32R, tag=f"wo{h}", name=f"wo{h}")
                  for h in range(NH)]
            for h in range(NH):
                nc.sync.dma_start(out=stg[h][:], in_=woT[h * P:(h + 1) * P, :])
            for ec in range(4):
                for h in range(NH):
                    nc.vector.tensor_copy(
                        wo[h][:, ec * 512:(ec + 1) * 512],
                        stg[h][:, ec * 512:(ec + 1) * 512])
            for qsb in range(4):
                zin = []
                for h in range(NH):
                    z32 = zinpool.tile([P, 512], F32R, tag=f"zin{h}", name=f"zin{h}")
                    if qsb == 3:
                        nc.vector.tensor_copy(z32[:], z3[h][:])
                    else:
                        zb = zinbpool.tile([P, 512], BF16, tag="zinb")
                        nc.sync.dma_start(
                            out=zb[:],
                            in_=zspill[h * P:(h + 1) * P, qsb * 512:(qsb + 1) * 512])
                        nc.vector.tensor_copy(z32[:], zb[:])
                    zin.append(z32)
                for ec in range(4):
                    po = [pacc.tile([P, 512], F32, tag="pacc", name=f"po{i}")
                          for i in range(4)]
                    for h in range(NH):
                        for qb in range(4):
                            nc.tensor.matmul(
                                po[qb][:],
                                zin[h][:, qb * P:(qb + 1) * P],
                                wo[h][:, ec * 512:(ec + 1) * 512],
                                start=(h == 0), stop=(h == NH - 1),
                            )
                    for qb in range(4):
                        ot = osb.tile([P, 512], F32, tag="ot")
                        nc.scalar.copy(ot[:], po[qb][:])
                        nc.sync.dma_start(
                            out=out[qsb * 512 + qb * P: qsb * 512 + (qb + 1) * P,
                                    ec * 512:(ec + 1) * 512],
                            in_=ot[:])

    nc.compile()
    return nc


def _host_tables():
    freqs = 1.0 / (THETA ** (np.arange(0, DH - 1, 2, dtype=np.float64) / DH))
    ang = np.arange(L, dtype=np.float64)[:, None] * freqs[None, :]  # (L, 64)
    cos = np.cos(ang)
    sin = np.sin(ang)
    cosT = np.empty((P, L), np.float32)
    sinT = np.empty((P, L), np.float32)
    cosT[0::2, :] = cos.T
    cosT[1::2, :] = cos.T
    sinT[0::2, :] = -sin.T
    sinT[1::2, :] = sin.T
    return cosT, sinT


def _host_masks():
    k = np.arange(P)[:, None]
    q = np.arange(256)[None, :]
    import ml_dtypes
    m = np.stack([
        (k <= q), (k <= q - 128), (k >= q + 1), (k >= q - 127),
    ]).astype(ml_dtypes.bfloat16)
    return m.reshape(4 * P, 256)


def _pack_core_inputs(x, Wq, Wk, Wv, Wo, n, g):
    """Prepacked per-core inputs; long contiguous per-partition DMA runs."""
    xT = np.ascontiguousarray(x[n].T)                      # (E, L)
    # xq[qtr*128+p, kt*512+c] = xT[kt*128+p, qtr*512+c]
    xq = xT.reshape(NKT, P, 4, 512).transpose(2, 1, 0, 3).reshape(4 * P, NKT * 512)
    # wqp[h*128+p, kt*128+c] = Wq.T[kt*128+p, g*1024+h*128+c]
    wqT = Wq[g * 1024:(g + 1) * 1024, :].T                 # (E, 1024)
    wqp = wqT.reshape(NKT, P, NH, DH).transpose(2, 1, 0, 3).reshape(NH * P, NKT * DH)
    # wkv[p, kt*512+j]: j<256 -> Wk.T slice, j>=256 -> Wv.T slice
    wkT = Wk[g * 256:(g + 1) * 256, :].T.reshape(NKT, P, 256)
    wvT = Wv[g * 256:(g + 1) * 256, :].T.reshape(NKT, P, 256)
    wkvp = np.concatenate([wkT, wvT], axis=2)              # (kt, p, 512)
    wkvp = wkvp.transpose(1, 0, 2).reshape(P, NKT * 512)
    woT = Wo[:, g * 1024:(g + 1) * 1024].T                 # (1024, E)
    import ml_dtypes
    return {
        "xq": np.ascontiguousarray(xq),
        "wqp": np.ascontiguousarray(wqp),
        "wkv": np.ascontiguousarray(wkvp),
        "woT": np.ascontiguousarray(woT).astype(ml_dtypes.bfloat16),
    }


def kernel(x, Wq, Wk, Wv, Wo):
    global _NC
    x = np.asarray(x, np.float32)
    Wq = np.asarray(Wq, np.float32)
    Wk = np.asarray(Wk, np.float32)
    Wv = np.asarray(Wv, np.float32)
    Wo = np.asarray(Wo, np.float32)

    if _NC is None:
        _NC = build_nc()
    nc = _NC

    cosT, sinT = _host_tables()
    masks = _host_masks()
    in_maps = []
    for c in range(8):
        n, g = c % 4, c // 4
        m = _pack_core_inputs(x, Wq, Wk, Wv, Wo, n, g)
        m.update({"cosT": cosT, "sinT": sinT, "masks": masks})
        in_maps.append(m)

    from concourse.bass_utils import run_bass_kernel_spmd
    res = run_bass_kernel_spmd(nc, in_maps, list(range(8)), trace=False)
    out = np.empty((N, L, E), np.float32)
    for n_ in range(4):
        out[n_] = res.results[n_]["out"] + res.results[4 + n_]["out"]
    return out


if __name__ == "__main__":
    rng = np.random.default_rng(0)
    x = rng.standard_normal((N, L, E), dtype=np.float32)
    Wq = (rng.standard_normal((E, E), dtype=np.float32) * 0.02)
    Wk = (rng.standard_normal((E // D, E), dtype=np.float32) * 0.02)
    Wv = (rng.standard_normal((E // D, E), dtype=np.float32) * 0.02)
    Wo = (rng.standard_normal((E, E), dtype=np.float32) * 0.02)
    print(kernel(x, Wq, Wk, Wv, Wo).shape)



# revision 23
# speedup vs baseline: 1.0276x; 1.0276x over previous
"""Sliding-window causal GQA attention (RoPE) for Trainium2, 8-core SPMD.

Problem: x:(4,2048,2048), Wq:(2048,2048), Wk/Wv:(512,2048), Wo:(2048,2048)
  q = rope(x @ Wq.T) 16 heads, k/v = (x @ Wk.T / x @ Wv.T) 4 kv heads (GQA x4),
  causal sliding-window attention (W=1024), out = z @ Wo.T.

Sharding: 8 cores = 4 batches x 2 head-groups (8 q-heads / 2 kv-heads each).
Each core computes a partial output (its head-group's Wo contribution) for its
batch; host sums the two partials per batch.

Per-core kernel (projections/scores in f32r; exp'd probabilities bf16):
  - layout: qT/kT as (head_dim, L) ["transposed"], v as (L, head_dim)
  - scores computed transposed S.T (keys on partitions, queries free) so P.T
    feeds the PV matmul directly with no on-chip transposes.
  - softmax denominator: ones[128,128] stationary matmul accumulates the
    per-query sum broadcast across all 128 partitions directly in PSUM
    (no separate M=1 sum + K=1 broadcast matmuls).
  - no max-subtraction in softmax: logits are O(1) here, exp is safe.
  - sliding window at 128-block granularity: query-super of 256 x up to 10
    key-blocks; boundary blocks masked via precomputed 0/1 tiles.
  - lag-2 software pipeline: the denominator/PV of super t are issued two
    score-slots later, so the PE never waits on exp/mask/RoPE latency.
  - inputs are host-prepacked so each DMA moves long contiguous runs per
    partition, keeping DMA packet counts low.
"""

import math
import numpy as np

H = 16
D = 4
WINDOW = 1024
THETA = 10000.0
N, L, E = 4, 2048, 2048
P = 128
DH = E // H          # 128 head dim
NH = H // 2          # 8 q heads per core
NKV = 2              # kv heads per core
NB = L // P          # 16 key blocks
NKT = E // P         # 16 contraction tiles
SCALE = 1.0 / math.sqrt(DH)

_NC = None


def _kbs_for_super(t):
    """Key blocks overlapping the window of query super t (256 queries)."""
    return list(range(max(0, 2 * t - 8), 2 * t + 2))


def build_nc():
    from contextlib import ExitStack
    from concourse import bacc, tile, mybir

    F32 = mybir.dt.float32
    F32R = mybir.dt.float32r
    BF16 = mybir.dt.bfloat16
    EXP = mybir.ActivationFunctionType.Exp

    SHUF_SWAP = [i ^ 1 for i in range(32)]

    nc = bacc.Bacc("TRN2", target_bir_lowering=False, debug=False)
    # prepacked inputs (see _pack_core_inputs for layouts)
    xq = nc.dram_tensor("xq", [4 * P, NKT * 512], F32R, kind="ExternalInput").ap()
    wqp = nc.dram_tensor("wqp", [NH * P, NKT * DH], F32R, kind="ExternalInput").ap()
    wkv = nc.dram_tensor("wkv", [P, NKT * 512], F32R, kind="ExternalInput").ap()
    woT = nc.dram_tensor("woT", [NH * DH, E], BF16, kind="ExternalInput").ap()
    cosT = nc.dram_tensor("cosT", [P, L], F32, kind="ExternalInput").ap()
    sinT = nc.dram_tensor("sinT", [P, L], F32, kind="ExternalInput").ap()
    masks = nc.dram_tensor("masks", [4 * P, 256], BF16, kind="ExternalInput").ap()
    out = nc.dram_tensor("out", [L, E], F32, kind="ExternalOutput").ap()
    zspill = nc.dram_tensor("zspill", [NH * P, L], BF16).ap()

    with tile.TileContext(nc) as tc, ExitStack() as stk:
        resid = stk.enter_context(tc.tile_pool(name="resid", bufs=1))
        kT = [resid.tile([P, L], BF16, tag=f"kT{i}", name=f"kT{i}") for i in range(NKV)]
        kvwc = [resid.tile([P, 2048], F32R, tag=f"kvw{dc}", name=f"kvw{dc}")
                for dc in range(4)]
        vt = [[resid.tile([P, P], BF16, tag=f"v{i}_{b}", name=f"v{i}_{b}") for b in range(NB)]
              for i in range(NKV)]

        z3 = [resid.tile([P, 512], BF16, tag=f"z3_{h}", name=f"z3_{h}")
              for h in range(NH)]
        # wo staging (bf16); loads issued near the end of attention so the
        # output projection never waits on them
        wostg = stk.enter_context(tc.tile_pool(name="wostg", bufs=1))
        stg = [wostg.tile([P, E], BF16, tag=f"wostg{h}", name=f"stg{h}")
               for h in range(NH)]
        const = stk.enter_context(tc.tile_pool(name="const", bufs=1))
        # mask kinds: 0=diagA (k<=q), 1=diagB (k<=q-128),
        #             2=farA (k>=q+1), 3=farB (k>=q-127)
        mk = [const.tile([P, 256], BF16, tag=f"mk{i}", name=f"mk{i}") for i in range(4)]
        ones_f = const.tile([P, P], F32, tag="ones_f")
        ones_r = const.tile([P, P], F32R, tag="ones_r")

        def rope_evict(dest, psum, cos_sl, sin_sl, tmp_pool, n):
            # dest = psum * cos + pairswap(psum) * sin   (sin pre-signed)
            tmp = tmp_pool.tile([P, 512], F32, tag="ropetmp", name="ropetmp")
            nc.vector.stream_shuffle(tmp[:, :n], psum, SHUF_SWAP)
            nc.vector.tensor_mul(tmp[:, :n], tmp[:, :n], sin_sl)
            nc.vector.tensor_mul(dest, psum, cos_sl)
            nc.vector.tensor_add(dest, dest, tmp[:, :n])

        osb = stk.enter_context(tc.tile_pool(name="osb", bufs=3))
        pacc = stk.enter_context(tc.tile_pool(name="pacc", bufs=5, space="PSUM"))
        pzp = stk.enter_context(tc.tile_pool(name="pz", bufs=2, space="PSUM"))
        pbp = stk.enter_context(tc.tile_pool(name="pb", bufs=1, space="PSUM"))
        with tc.tile_pool(name="quarter", bufs=2) as qpool, \
             tc.tile_pool(name="wq", bufs=2) as wqpool, \
             tc.tile_pool(name="work", bufs=3) as work, \
             tc.tile_pool(name="qt", bufs=2) as qtpool, \
             tc.tile_pool(name="zev", bufs=2) as zevpool, \
             tc.tile_pool(name="scr", bufs=1) as scrpool, \
             tc.tile_pool(name="rtmp", bufs=1) as rtmp:

            def load_quarter(qtr):
                xtc = [qpool.tile([P, 2048], F32R, tag=f"xt{dc}", name=f"xt{dc}")
                       for dc in range(4)]
                cos_q = qpool.tile([P, 512], F32, tag="cos", bufs=1)
                sin_q = qpool.tile([P, 512], F32, tag="sin", bufs=1)
                for dc in range(4):
                    nc.sync.dma_start(
                        out=xtc[dc][:],
                        in_=xq[qtr * P:(qtr + 1) * P, dc * 2048:(dc + 1) * 2048])
                c0 = 512 * qtr
                nc.sync.dma_start(out=cos_q[:], in_=cosT[:, c0:c0 + 512])
                nc.sync.dma_start(out=sin_q[:], in_=sinT[:, c0:c0 + 512])
                return xtc, cos_q, sin_q

            # startup: interleave kvw/x chunks in consumption order so the
            # first K-proj matmuls start after ~2MB instead of ~8MB
            xtc0 = [qpool.tile([P, 2048], F32R, tag=f"xt{dc}", name=f"xt{dc}")
                    for dc in range(4)]
            cos_q0 = qpool.tile([P, 512], F32, tag="cos", bufs=1)
            sin_q0 = qpool.tile([P, 512], F32, tag="sin", bufs=1)
            for dc in range(4):
                nc.sync.dma_start(out=kvwc[dc][:],
                                  in_=wkv[:, dc * 2048:(dc + 1) * 2048])
                nc.sync.dma_start(out=xtc0[dc][:],
                                  in_=xq[:P, dc * 2048:(dc + 1) * 2048])
                if dc == 0:
                    nc.sync.dma_start(out=cos_q0[:], in_=cosT[:, :512])
                    nc.sync.dma_start(out=sin_q0[:], in_=sinT[:, :512])
            cur = (xtc0, cos_q0, sin_q0)

            # rolling Wq prefetch, 3 heads deep (global head index)
            wq_q = []

            def wq_prefetch(g):
                if g >= 4 * NH:
                    return
                h = g % NH
                wqt = wqpool.tile([P, NKT * DH], F32R, tag="wqh", name="wqt")
                nc.sync.dma_start(out=wqt[:], in_=wqp[h * P:(h + 1) * P, :])
                wq_q.append(wqt)

            for i in range(4):
                nc.sync.dma_start(out=mk[i][:], in_=masks[i * P:(i + 1) * P, :])
            nc.vector.memset(ones_f[:], 1.0)
            nc.vector.tensor_copy(ones_r[:], ones_f[:])

            # lag-2 pipeline of attention tails
            pend = []

            def attn_tail():
                kv, kbs, pt, h, t = pend.pop(0)
                nkb = len(kbs)

                def qspan(kb):
                    # diagB (kb==2t+1) is all-masked for the first 128 queries
                    # of the super; farA (kb==2t-8) for the last 128. Skip the
                    # dead half in the PV matmuls (partial-width PSUM
                    # accumulation zero-fills untouched columns).
                    if kb == 2 * t + 1:
                        return 128, 256
                    if kb == 2 * t - 8:
                        return 0, 128
                    return 0, 256

                # PV
                pz = pzp.tile([P, 256], F32, tag="pz")
                for i, kb in enumerate(kbs):
                    a, b = qspan(kb)
                    nc.tensor.matmul(
                        pz[:, a:b], vt[kv][kb][:],
                        pt[:, i * 256 + a:i * 256 + b],
                        start=(i == 0), stop=(i == nkb - 1))
                # softmax denominator: fold the key-block dim on DVE (f32
                # tree of pairwise adds; dead boundary halves are zeroed by
                # the masks), then a single ones-matmul broadcasts the
                # cross-partition sum into PSUM.
                ksum = zevpool.tile([P, 256], F32R, tag="ksum")
                if nkb == 2:
                    nc.vector.tensor_add(ksum[:], pt[:, 0:256], pt[:, 256:512])
                else:
                    scr = scrpool.tile([P, 1280], BF16, tag="scr")
                    hw = (nkb // 2) * 256
                    nc.vector.tensor_add(scr[:, :hw], pt[:, :hw], pt[:, hw:2 * hw])
                    if nkb == 4:
                        nc.vector.tensor_add(ksum[:], scr[:, 0:256], scr[:, 256:512])
                    elif nkb == 6:
                        nc.vector.tensor_add(scr[:, 0:256], scr[:, 0:256], scr[:, 256:512])
                        nc.vector.tensor_add(ksum[:], scr[:, 0:256], scr[:, 512:768])
                    elif nkb == 8:
                        nc.vector.tensor_add(scr[:, 0:512], scr[:, 0:512], scr[:, 512:1024])
                        nc.vector.tensor_add(ksum[:], scr[:, 0:256], scr[:, 256:512])
                    else:  # nkb == 10
                        nc.vector.tensor_add(scr[:, 0:512], scr[:, 0:512], scr[:, 512:1024])
                        nc.vector.tensor_add(scr[:, 0:256], scr[:, 0:256], scr[:, 256:512])
                        nc.vector.tensor_add(ksum[:], scr[:, 0:256], scr[:, 1024:1280])
                pb = pbp.tile([P, 256], F32, tag="pb")
                nc.tensor.matmul(pb[:], ones_r[:], ksum[:], start=True, stop=True)
                rec = zevpool.tile([P, 256], F32, tag="rec")
                nc.vector.reciprocal_approx_fast(rec[:], pb[:])
                if t >= 6:
                    s3 = t - 6
                    nc.vector.tensor_mul(
                        z3[h][:, s3 * 256:(s3 + 1) * 256], pz[:], rec[:])
                else:
                    zev = zevpool.tile([P, 256], BF16, tag="zev")
                    nc.vector.tensor_mul(zev[:], pz[:], rec[:])
                    nc.sync.dma_start(
                        out=zspill[h * P:(h + 1) * P, t * 256:(t + 1) * 256],
                        in_=zev[:])

            for qtr in range(4):
                xt, cos_q, sin_q = cur

                def xtile(kt, a, b):
                    return xt[kt // 4][:, (kt % 4) * 512 + a: (kt % 4) * 512 + b]

                def kvw_sl(kt, a, b):
                    return kvwc[kt // 4][:, (kt % 4) * 512 + a: (kt % 4) * 512 + b]

                c0 = 512 * qtr

                # K projection (+RoPE) for both kv heads
                for kv in range(NKV):
                    pk = pacc.tile([P, 512], F32, tag="pacc")
                    for kt in range(NKT):
                        nc.tensor.matmul(
                            pk[:],
                            kvw_sl(kt, kv * DH, (kv + 1) * DH),
                            xtile(kt, 0, 512),
                            start=(kt == 0), stop=(kt == NKT - 1),
                        )
                    rope_evict(kT[kv][:, c0:c0 + 512], pk[:], cos_q[:], sin_q[:], rtmp, 512)

                if qtr == 0:
                    # deferred so these 2MB don't share DMA bandwidth with
                    # the critical kvw+xt startup burst
                    for g0 in range(2):
                        wq_prefetch(g0)

                # V projection (both kv heads at once, natural layout)
                for lb in range(4):
                    pv = pacc.tile([P, 512], F32, tag="pacc")
                    for kt in range(NKT):
                        nc.tensor.matmul(
                            pv[:, :NKV * DH],
                            xtile(kt, lb * P, (lb + 1) * P),
                            kvw_sl(kt, 256, 512),
                            start=(kt == 0), stop=(kt == NKT - 1),
                        )
                    for kv in range(NKV):
                        nc.scalar.copy(vt[kv][4 * qtr + lb][:], pv[:, kv * DH:(kv + 1) * DH])

                def q_proj(g):
                    """Q projection + RoPE for one head; returns the roped tile."""
                    wq = wq_q.pop(0)
                    wq_prefetch(g + 2)
                    pq = pacc.tile([P, 512], F32, tag="pacc")
                    for kt in range(NKT):
                        nc.tensor.matmul(
                            pq[:],
                            wq[:, kt * DH:(kt + 1) * DH],
                            xtile(kt, 0, 512),
                            start=(kt == 0), stop=(kt == NKT - 1),
                        )
                    qth = qtpool.tile([P, 512], BF16, tag="qt")
                    rope_evict(qth[:], pq[:], cos_q[:], sin_q[:], rtmp, 512)
                    return qth

                # Q projection + attention, head-major; Q runs one head ahead
                # so RoPE latency is always covered by PE work
                qnext = q_proj(qtr * NH)
                for h in range(NH):
                    kv = h // (NH // NKV)
                    qth = qnext
                    if h + 1 < NH:
                        qnext = q_proj(qtr * NH + h + 1)
                    if h == 1 and qtr < 3:
                        cur = load_quarter(qtr + 1)
                    if h == 1 and qtr == 3:
                        # prefetch wo for the output projection; by now the
                        # DMA engines are mostly idle
                        for hh in range(NH):
                            nc.sync.dma_start(out=stg[hh][:],
                                              in_=woT[hh * P:(hh + 1) * P, :])
                    for s in range(2):
                        t = 2 * qtr + s
                        qt = qth[:, s * 256:(s + 1) * 256]

                        # drain the oldest pending super (lag 2)
                        if len(pend) >= 2:
                            attn_tail()

                        kbs = _kbs_for_super(t)
                        nkb = len(kbs)
                        pt = work.tile([P, 2560], BF16, tag="pt")
                        # scores (transposed: keys on partitions) in chunks of 2 kb
                        for ci in range(0, nkb, 2):
                            cn = min(2, nkb - ci)
                            ps = pacc.tile([P, 512], F32, tag="pacc")
                            for i in range(cn):
                                kb = kbs[ci + i]
                                nc.tensor.matmul(
                                    ps[:, i * 256:(i + 1) * 256],
                                    kT[kv][:, kb * P:(kb + 1) * P],
                                    qt,
                                    start=True, stop=True,
                                )
                            nc.scalar.activation(
                                pt[:, ci * 256:(ci + cn) * 256],
                                ps[:, :cn * 256], EXP, scale=SCALE)
                        # window masks on boundary blocks. diagB/farA are
                        # masked full-width (their masks are 0 on the dead
                        # half) so the denominator tree can sum whole blocks;
                        # the PV matmuls still skip the dead halves.
                        for i, kb in enumerate(kbs):
                            if kb == 2 * t:
                                kind, a, b = 0, 0, 256
                            elif kb == 2 * t + 1:
                                kind, a, b = 1, 0, 256
                            elif kb == 2 * t - 8:
                                kind, a, b = 2, 0, 256
                            elif kb == 2 * t - 7:
                                kind, a, b = 3, 0, 256
                            else:
                                continue
                            sl = pt[:, i * 256 + a:i * 256 + b]
                            nc.vector.tensor_mul(sl, sl, mk[kind][:, a:b])
                        pend.append((kv, kbs, pt, h, t))

            while pend:
                attn_tail()

        # Output projection: out[q,:] += sum_h zTn_h[:,q].T @ woT[h]
        # wo and z are bf16 and feed the matmuls directly (bf16 matmul runs
        # at the same rate as f32r; the data was bf16-quantized anyway).
        with tc.tile_pool(name="zinb", bufs=16) as zinbpool:
            def load_z(qsb):
                if qsb == 3:
                    return z3
                zin = []
                for h in range(NH):
                    zb = zinbpool.tile([P, 512], BF16, tag="zinb")
                    nc.sync.dma_start(
                        out=zb[:],
                        in_=zspill[h * P:(h + 1) * P, qsb * 512:(qsb + 1) * 512])
                    zin.append(zb)
                return zin

            znext = load_z(0)
            for qsb in range(4):
                zin = znext
                if qsb < 3:
                    znext = load_z(qsb + 1)
                for ec in range(4):
                    po = [pacc.tile([P, 512], F32, tag="pacc", name=f"po{i}")
                          for i in range(4)]
                    for h in range(NH):
                        for qb in range(4):
                            nc.tensor.matmul(
                                po[qb][:],
                                zin[h][:, qb * P:(qb + 1) * P],
                                stg[h][:, ec * 512:(ec + 1) * 512],
                                start=(h == 0), stop=(h == NH - 1),
                            )
                    for qb in range(4):
                        for cc in range(2):
                            ot = osb.tile([P, 256], F32, tag="ot")
                            nc.scalar.copy(ot[:], po[qb][:, cc * 256:(cc + 1) * 256])
                            nc.sync.dma_start(
                                out=out[qsb * 512 + qb * P: qsb * 512 + (qb + 1) * P,
                                        ec * 512 + cc * 256: ec * 512 + (cc + 1) * 256],
                                in_=ot[:])

    nc.compile()
    return nc


def _host_tables():
    freqs = 1.0 / (THETA ** (np.arange(0, DH - 1, 2, dtype=np.float64) / DH))
    ang = np.arange(L, dtype=np.float64)[:, None] * freqs[None, :]  # (L, 64)
    cos = np.cos(ang)
    sin = np.sin(ang)
    cosT = np.empty((P, L), np.float32)
    sinT = np.empty((P, L), np.float32)
    cosT[0::2, :] = cos.T
    cosT[1::2, :] = cos.T
    sinT[0::2, :] = -sin.T
    sinT[1::2, :] = sin.T
    return cosT, sinT


def _host_masks():
    k = np.arange(P)[:, None]
    q = np.arange(256)[None, :]
    import ml_dtypes
    m = np.stack([
        (k <= q), (k <= q - 128), (k >= q + 1), (k >= q - 127),
    ]).astype(ml_dtypes.bfloat16)
    return m.reshape(4 * P, 256)


def _pack_core_inputs(x, Wq, Wk, Wv, Wo, n, g):
    """Prepacked per-core inputs; long contiguous per-partition DMA runs."""
    xT = np.ascontiguousarray(x[n].T)                      # (E, L)
    # xq[qtr*128+p, kt*512+c] = xT[kt*128+p, qtr*512+c]
    xq = xT.reshape(NKT, P, 4, 512).transpose(2, 1, 0, 3).reshape(4 * P, NKT * 512)
    # wqp[h*128+p, kt*128+c] = Wq.T[kt*128+p, g*1024+h*128+c]
    wqT = Wq[g * 1024:(g + 1) * 1024, :].T                 # (E, 1024)
    wqp = wqT.reshape(NKT, P, NH, DH).transpose(2, 1, 0, 3).reshape(NH * P, NKT * DH)
    # wkv[p, kt*512+j]: j<256 -> Wk.T slice, j>=256 -> Wv.T slice
    wkT = Wk[g * 256:(g + 1) * 256, :].T.reshape(NKT, P, 256)
    wvT = Wv[g * 256:(g + 1) * 256, :].T.reshape(NKT, P, 256)
    wkvp = np.concatenate([wkT, wvT], axis=2)              # (kt, p, 512)
    wkvp = wkvp.transpose(1, 0, 2).reshape(P, NKT * 512)
    woT = Wo[:, g * 1024:(g + 1) * 1024].T                 # (1024, E)
    import ml_dtypes
    return {
        "xq": np.ascontiguousarray(xq),
        "wqp": np.ascontiguousarray(wqp),
        "wkv": np.ascontiguousarray(wkvp),
        "woT": np.ascontiguousarray(woT).astype(ml_dtypes.bfloat16),
    }


def kernel(x, Wq, Wk, Wv, Wo):
    global _NC
    x = np.asarray(x, np.float32)
    Wq = np.asarray(Wq, np.float32)
    Wk = np.asarray(Wk, np.float32)
    Wv = np.asarray(Wv, np.float32)
    Wo = np.asarray(Wo, np.float32)

    if _NC is None:
        _NC = build_nc()
    nc = _NC

    cosT, sinT = _host_tables()
    masks = _host_masks()
    in_maps = []
    for c in range(8):
        n, g = c % 4, c // 4
        m = _pack_core_inputs(x, Wq, Wk, Wv, Wo, n, g)
        m.update({"cosT": cosT, "sinT": sinT, "masks": masks})
        in_maps.append(m)

    from concourse.bass_utils import run_bass_kernel_spmd
    res = run_bass_kernel_spmd(nc, in_maps, list(range(8)), trace=False)
    out = np.empty((N, L, E), np.float32)
    for n_ in range(4):
        out[n_] = res.results[n_]["out"] + res.results[4 + n_]["out"]
    return out


if __name__ == "__main__":
    rng = np.random.default_rng(0)
    x = rng.standard_normal((N, L, E), dtype=np.float32)
    Wq = (rng.standard_normal((E, E), dtype=np.float32) * 0.02)
    Wk = (rng.standard_normal((E // D, E), dtype=np.float32) * 0.02)
    Wv = (rng.standard_normal((E // D, E), dtype=np.float32) * 0.02)
    Wo = (rng.standard_normal((E, E), dtype=np.float32) * 0.02)
    print(kernel(x, Wq, Wk, Wv, Wo).shape)



# revision 40
# speedup vs baseline: 1.1682x; 1.1368x over previous
"""Sliding-window causal GQA attention (RoPE) for Trainium2, 8-core SPMD.

Problem: x:(4,2048,2048), Wq:(2048,2048), Wk/Wv:(512,2048), Wo:(2048,2048)
  q = rope(x @ Wq.T) 16 heads, k/v = (x @ Wk.T / x @ Wv.T) 4 kv heads (GQA x4),
  causal sliding-window attention (W=1024), out = z @ Wo.T.

Sharding: 8 cores = 4 batches x 2 head-groups (8 q-heads / 2 kv-heads each).
Each core computes a partial output (its head-group's Wo contribution) for its
batch; host sums the two partials per batch.

Per-core kernel (projections/scores in f32r; exp'd probabilities bf16):
  - layout: qT/kT as (head_dim, L) ["transposed"], v as (L, head_dim)
  - scores computed transposed S.T (keys on partitions, queries free) so P.T
    feeds the PV matmul directly with no on-chip transposes.
  - softmax denominator: ones[128,128] stationary matmul accumulates the
    per-query sum broadcast across all 128 partitions directly in PSUM
    (no separate M=1 sum + K=1 broadcast matmuls).
  - no max-subtraction in softmax: logits are O(1) here, exp is safe.
  - sliding window at 128-block granularity: query-super of 256 x up to 10
    key-blocks; boundary blocks masked via precomputed 0/1 tiles.
  - lag-2 software pipeline: the denominator/PV of super t are issued two
    score-slots later, so the PE never waits on exp/mask/RoPE latency.
  - inputs are host-prepacked so each DMA moves long contiguous runs per
    partition, keeping DMA packet counts low.
"""

import math
import numpy as np

H = 16
D = 4
WINDOW = 1024
THETA = 10000.0
N, L, E = 4, 2048, 2048
P = 128
DH = E // H          # 128 head dim
NH = H // 2          # 8 q heads per core
NKV = 2              # kv heads per core
NB = L // P          # 16 key blocks
NKT = E // P         # 16 contraction tiles
SCALE = 1.0 / math.sqrt(DH)

_NC = None


def _kbs_for_super(t):
    """Key blocks overlapping the window of query super t (256 queries)."""
    return list(range(max(0, 2 * t - 8), 2 * t + 2))


def build_nc():
    from contextlib import ExitStack
    from concourse import bacc, tile, mybir

    F32 = mybir.dt.float32
    F32R = mybir.dt.float32r
    BF16 = mybir.dt.bfloat16
    EXP = mybir.ActivationFunctionType.Exp

    SHUF_SWAP = [i ^ 1 for i in range(32)]

    nc = bacc.Bacc("TRN2", target_bir_lowering=False, debug=False)
    # prepacked inputs (see _pack_core_inputs for layouts); x and the
    # projection weights ship bf16 (halves DMA; softmax cancels most of
    # the logit-path quantization noise)
    xq = nc.dram_tensor("xq", [4 * P, NKT * 512], BF16, kind="ExternalInput").ap()
    wqp = nc.dram_tensor("wqp", [NH * P, NKT * DH], BF16, kind="ExternalInput").ap()
    wkv = nc.dram_tensor("wkv", [P, NKT * 512], BF16, kind="ExternalInput").ap()
    woT = nc.dram_tensor("woT", [NH * DH, E], BF16, kind="ExternalInput").ap()
    cosT = nc.dram_tensor("cosT", [P, L], F32, kind="ExternalInput").ap()
    sinT = nc.dram_tensor("sinT", [P, L], F32, kind="ExternalInput").ap()
    masks = nc.dram_tensor("masks", [4 * P, 256], BF16, kind="ExternalInput").ap()
    out = nc.dram_tensor("out", [L, E], F32, kind="ExternalOutput").ap()
    zspill = nc.dram_tensor("zspill", [NH * P, L], BF16).ap()

    with tile.TileContext(nc) as tc, ExitStack() as stk:
        resid = stk.enter_context(tc.tile_pool(name="resid", bufs=1))
        kT = [resid.tile([P, L], BF16, tag=f"kT{i}", name=f"kT{i}") for i in range(NKV)]
        kvwc = [resid.tile([P, 2048], BF16, tag=f"kvw{dc}", name=f"kvw{dc}")
                for dc in range(4)]
        vt = [[resid.tile([P, P], BF16, tag=f"v{i}_{b}", name=f"v{i}_{b}") for b in range(NB)]
              for i in range(NKV)]

        z3 = [resid.tile([P, 512], BF16, tag=f"z3_{h}", name=f"z3_{h}")
              for h in range(NH)]
        # wo staging (bf16); loads issued near the end of attention so the
        # output projection never waits on them
        wostg = stk.enter_context(tc.tile_pool(name="wostg", bufs=1))
        stg = [wostg.tile([P, E], BF16, tag=f"wostg{h}", name=f"stg{h}")
               for h in range(NH)]
        const = stk.enter_context(tc.tile_pool(name="const", bufs=1))
        # mask kinds: 0=diagA (k<=q), 1=diagB (k<=q-128),
        #             2=farA (k>=q+1), 3=farB (k>=q-127)
        mk = [const.tile([P, 256], BF16, tag=f"mk{i}", name=f"mk{i}") for i in range(4)]
        ones_f = const.tile([P, P], F32, tag="ones_f")
        ones = const.tile([P, P], BF16, tag="ones")

        def rope_evict(dest, psum, cos_sl, sin_sl, tmp_pool, n):
            # dest = psum * cos + pairswap(psum) * sin   (sin pre-signed)
            tmp = tmp_pool.tile([P, 512], F32, tag="ropetmp", name="ropetmp")
            nc.vector.stream_shuffle(tmp[:, :n], psum, SHUF_SWAP)
            nc.vector.tensor_mul(tmp[:, :n], tmp[:, :n], sin_sl)
            nc.vector.tensor_mul(dest, psum, cos_sl)
            nc.vector.tensor_add(dest, dest, tmp[:, :n])

        osb = stk.enter_context(tc.tile_pool(name="osb", bufs=8))
        with tc.tile_pool(name="pacc", bufs=5, space="PSUM") as pacc, \
             tc.tile_pool(name="pz", bufs=2, space="PSUM") as pzp, \
             tc.tile_pool(name="pb", bufs=1, space="PSUM") as pbp, \
             tc.tile_pool(name="quarter", bufs=2) as qpool, \
             tc.tile_pool(name="wq", bufs=3) as wqpool, \
             tc.tile_pool(name="work", bufs=4) as work, \
             tc.tile_pool(name="qt", bufs=2) as qtpool, \
             tc.tile_pool(name="zev", bufs=3) as zevpool, \
             tc.tile_pool(name="scr", bufs=2) as scrpool, \
             tc.tile_pool(name="rtmp", bufs=2) as rtmp:

            def load_quarter(qtr):
                xtc = [qpool.tile([P, 2048], BF16, tag=f"xt{dc}", name=f"xt{dc}")
                       for dc in range(4)]
                cos_q = qpool.tile([P, 512], F32, tag="cos")
                sin_q = qpool.tile([P, 512], F32, tag="sin")
                for dc in range(4):
                    nc.sync.dma_start(
                        out=xtc[dc][:],
                        in_=xq[qtr * P:(qtr + 1) * P, dc * 2048:(dc + 1) * 2048])
                c0 = 512 * qtr
                nc.sync.dma_start(out=cos_q[:], in_=cosT[:, c0:c0 + 512])
                nc.sync.dma_start(out=sin_q[:], in_=sinT[:, c0:c0 + 512])
                return xtc, cos_q, sin_q

            # startup: interleave kvw/x chunks in consumption order so the
            # first K-proj matmuls start after ~2MB instead of ~8MB
            xtc0 = [qpool.tile([P, 2048], BF16, tag=f"xt{dc}", name=f"xt{dc}")
                    for dc in range(4)]
            cos_q0 = qpool.tile([P, 512], F32, tag="cos")
            sin_q0 = qpool.tile([P, 512], F32, tag="sin")
            for dc in range(4):
                nc.sync.dma_start(out=kvwc[dc][:],
                                  in_=wkv[:, dc * 2048:(dc + 1) * 2048])
                nc.sync.dma_start(out=xtc0[dc][:],
                                  in_=xq[:P, dc * 2048:(dc + 1) * 2048])
                if dc == 0:
                    nc.sync.dma_start(out=cos_q0[:], in_=cosT[:, :512])
                    nc.sync.dma_start(out=sin_q0[:], in_=sinT[:, :512])
            cur = (xtc0, cos_q0, sin_q0)

            # rolling Wq prefetch, 3 heads deep (global head index)
            wq_q = []

            def wq_prefetch(g):
                if g >= 4 * NH:
                    return
                h = g % NH
                wqt = wqpool.tile([P, NKT * DH], BF16, tag="wqh", name="wqt")
                nc.sync.dma_start(out=wqt[:], in_=wqp[h * P:(h + 1) * P, :])
                wq_q.append(wqt)

            for i in range(4):
                nc.sync.dma_start(out=mk[i][:], in_=masks[i * P:(i + 1) * P, :])
            nc.vector.memset(ones_f[:], 1.0)
            nc.vector.tensor_copy(ones[:], ones_f[:])

            # lag-2 pipeline of attention tails
            pend = []

            def attn_tail():
                kv, kbs, pt, h, t = pend.pop(0)
                nkb = len(kbs)

                def qspan(kb):
                    # diagB (kb==2t+1) is all-masked for the first 128 queries
                    # of the super; farA (kb==2t-8) for the last 128. Skip the
                    # dead half in the PV matmuls (partial-width PSUM
                    # accumulation zero-fills untouched columns).
                    if kb == 2 * t + 1:
                        return 128, 256
                    if kb == 2 * t - 8:
                        return 0, 128
                    return 0, 256

                # PV
                pz = pzp.tile([P, 256], F32, tag="pz")
                for i, kb in enumerate(kbs):
                    a, b = qspan(kb)
                    nc.tensor.matmul(
                        pz[:, a:b], vt[kv][kb][:],
                        pt[:, i * 256 + a:i * 256 + b],
                        start=(i == 0), stop=(i == nkb - 1))
                # softmax denominator: fold the key-block dim on DVE (f32
                # tree of pairwise adds; dead boundary halves are zeroed by
                # the masks), then a single ones-matmul broadcasts the
                # cross-partition sum into PSUM.
                ksum = zevpool.tile([P, 256], BF16, tag="ksum")
                if nkb == 2:
                    nc.vector.tensor_add(ksum[:], pt[:, 0:256], pt[:, 256:512])
                else:
                    scr = scrpool.tile([P, 1280], BF16, tag="scr")
                    hw = (nkb // 2) * 256
                    nc.vector.tensor_add(scr[:, :hw], pt[:, :hw], pt[:, hw:2 * hw])
                    if nkb == 4:
                        nc.vector.tensor_add(ksum[:], scr[:, 0:256], scr[:, 256:512])
                    elif nkb == 6:
                        nc.vector.tensor_add(scr[:, 0:256], scr[:, 0:256], scr[:, 256:512])
                        nc.vector.tensor_add(ksum[:], scr[:, 0:256], scr[:, 512:768])
                    elif nkb == 8:
                        nc.vector.tensor_add(scr[:, 0:512], scr[:, 0:512], scr[:, 512:1024])
                        nc.vector.tensor_add(ksum[:], scr[:, 0:256], scr[:, 256:512])
                    else:  # nkb == 10
                        nc.vector.tensor_add(scr[:, 0:512], scr[:, 0:512], scr[:, 512:1024])
                        nc.vector.tensor_add(scr[:, 0:256], scr[:, 0:256], scr[:, 256:512])
                        nc.vector.tensor_add(ksum[:], scr[:, 0:256], scr[:, 1024:1280])
                pb = pbp.tile([P, 256], F32, tag="pb")
                nc.tensor.matmul(pb[:], ones[:], ksum[:], start=True, stop=True)
                rec = zevpool.tile([P, 256], F32, tag="rec")
                nc.vector.reciprocal_approx_fast(rec[:], pb[:])
                if t >= 6:
                    s3 = t - 6
                    nc.vector.tensor_mul(
                        z3[h][:, s3 * 256:(s3 + 1) * 256], pz[:], rec[:])
                else:
                    zev = zevpool.tile([P, 256], BF16, tag="zev")
                    nc.vector.tensor_mul(zev[:], pz[:], rec[:])
                    nc.sync.dma_start(
                        out=zspill[h * P:(h + 1) * P, t * 256:(t + 1) * 256],
                        in_=zev[:])

            for qtr in range(4):
                xt, cos_q, sin_q = cur

                def xtile(kt, a, b):
                    return xt[kt // 4][:, (kt % 4) * 512 + a: (kt % 4) * 512 + b]

                def kvw_sl(kt, a, b):
                    return kvwc[kt // 4][:, (kt % 4) * 512 + a: (kt % 4) * 512 + b]

                c0 = 512 * qtr

                # K projection (+RoPE) for both kv heads
                for kv in range(NKV):
                    pk = pacc.tile([P, 512], F32, tag="pacc")
                    for kt in range(NKT):
                        nc.tensor.matmul(
                            pk[:],
                            kvw_sl(kt, kv * DH, (kv + 1) * DH),
                            xtile(kt, 0, 512),
                            start=(kt == 0), stop=(kt == NKT - 1),
                        )
                    rope_evict(kT[kv][:, c0:c0 + 512], pk[:], cos_q[:], sin_q[:], rtmp, 512)

                if qtr == 0:
                    # deferred so these don't share DMA bandwidth with the
                    # critical kvw+xt startup burst
                    for g0 in range(3):
                        wq_prefetch(g0)

                # V projection (both kv heads at once, natural layout)
                for lb in range(4):
                    pv = pacc.tile([P, 512], F32, tag="pacc")
                    for kt in range(NKT):
                        nc.tensor.matmul(
                            pv[:, :NKV * DH],
                            xtile(kt, lb * P, (lb + 1) * P),
                            kvw_sl(kt, 256, 512),
                            start=(kt == 0), stop=(kt == NKT - 1),
                        )
                    for kv in range(NKV):
                        nc.scalar.copy(vt[kv][4 * qtr + lb][:], pv[:, kv * DH:(kv + 1) * DH])

                def q_proj(g):
                    """Q projection + RoPE for one head; returns the roped tile."""
                    wq = wq_q.pop(0)
                    wq_prefetch(g + 3)
                    pq = pacc.tile([P, 512], F32, tag="pacc")
                    for kt in range(NKT):
                        nc.tensor.matmul(
                            pq[:],
                            wq[:, kt * DH:(kt + 1) * DH],
                            xtile(kt, 0, 512),
                            start=(kt == 0), stop=(kt == NKT - 1),
                        )
                    qth = qtpool.tile([P, 512], BF16, tag="qt")
                    rope_evict(qth[:], pq[:], cos_q[:], sin_q[:], rtmp, 512)
                    return qth

                # Q projection + attention, head-major; Q runs one head ahead
                # so RoPE latency is always covered by PE work
                qnext = q_proj(qtr * NH)
                for h in range(NH):
                    kv = h // (NH // NKV)
                    qth = qnext
                    if h + 1 < NH:
                        qnext = q_proj(qtr * NH + h + 1)
                    if h == 1 and qtr < 3:
                        cur = load_quarter(qtr + 1)
                    if h == 1 and qtr == 3:
                        # prefetch wo for the output projection; by now the
                        # DMA engines are mostly idle
                        for hh in range(NH):
                            nc.sync.dma_start(out=stg[hh][:],
                                              in_=woT[hh * P:(hh + 1) * P, :])
                    for s in range(2):
                        t = 2 * qtr + s
                        qt = qth[:, s * 256:(s + 1) * 256]

                        # drain the oldest pending super (lag 2)
                        if len(pend) >= 2:
                            attn_tail()

                        kbs = _kbs_for_super(t)
                        nkb = len(kbs)
                        pt = work.tile([P, 2560], BF16, tag="pt")
                        # scores (transposed: keys on partitions) in chunks of
                        # 2 kb; dead query-halves of the boundary blocks are
                        # skipped (untouched PSUM reads back as zero, and the
                        # full-width masks zero those slots after exp)
                        for ci in range(0, nkb, 2):
                            cn = min(2, nkb - ci)
                            ps = pacc.tile([P, 512], F32, tag="pacc")
                            for i in range(cn):
                                kb = kbs[ci + i]
                                if kb == 2 * t + 1:
                                    a, b = 128, 256
                                elif kb == 2 * t - 8:
                                    a, b = 0, 128
                                else:
                                    a, b = 0, 256
                                nc.tensor.matmul(
                                    ps[:, i * 256 + a:i * 256 + b],
                                    kT[kv][:, kb * P:(kb + 1) * P],
                                    qt[:, a:b],
                                    start=True, stop=True,
                                )
                            nc.scalar.activation(
                                pt[:, ci * 256:(ci + cn) * 256],
                                ps[:, :cn * 256], EXP, scale=SCALE)
                        # window masks on boundary blocks. diagB/farA are
                        # masked full-width (their masks are 0 on the dead
                        # half) so the denominator tree can sum whole blocks;
                        # the PV matmuls still skip the dead halves.
                        for i, kb in enumerate(kbs):
                            if kb == 2 * t:
                                kind, a, b = 0, 0, 256
                            elif kb == 2 * t + 1:
                                kind, a, b = 1, 0, 256
                            elif kb == 2 * t - 8:
                                kind, a, b = 2, 0, 256
                            elif kb == 2 * t - 7:
                                kind, a, b = 3, 0, 256
                            else:
                                continue
                            sl = pt[:, i * 256 + a:i * 256 + b]
                            nc.vector.tensor_mul(sl, sl, mk[kind][:, a:b])
                        pend.append((kv, kbs, pt, h, t))

            while pend:
                attn_tail()

        # Output projection: out[q,:] += sum_h zTn_h[:,q].T @ woT[h]
        # wo and z are bf16 and feed the matmuls directly (bf16 matmul runs
        # at the same rate as f32r; the data was bf16-quantized anyway).
        with tc.tile_pool(name="zinb", bufs=16) as zinbpool, \
             tc.tile_pool(name="po", bufs=8, space="PSUM") as pop:
            def load_z(qsb):
                if qsb == 3:
                    return z3
                zin = []
                for h in range(NH):
                    zb = zinbpool.tile([P, 512], BF16, tag="zinb")
                    nc.sync.dma_start(
                        out=zb[:],
                        in_=zspill[h * P:(h + 1) * P, qsb * 512:(qsb + 1) * 512])
                    zin.append(zb)
                return zin

            znext = load_z(0)
            for qsb in range(4):
                zin = znext
                if qsb < 3:
                    znext = load_z(qsb + 1)
                for ec in range(4):
                    po = [pop.tile([P, 512], F32, tag="po", name=f"po{i}")
                          for i in range(4)]
                    for h in range(NH):
                        for qb in range(4):
                            nc.tensor.matmul(
                                po[qb][:],
                                zin[h][:, qb * P:(qb + 1) * P],
                                stg[h][:, ec * 512:(ec + 1) * 512],
                                start=(h == 0), stop=(h == NH - 1),
                            )
                    for qb in range(4):
                        for cc in range(2):
                            ot = osb.tile([P, 256], F32, tag="ot")
                            nc.scalar.copy(ot[:], po[qb][:, cc * 256:(cc + 1) * 256])
                            nc.sync.dma_start(
                                out=out[qsb * 512 + qb * P: qsb * 512 + (qb + 1) * P,
                                        ec * 512 + cc * 256: ec * 512 + (cc + 1) * 256],
                                in_=ot[:])

    nc.compile()
    return nc


def _host_tables():
    freqs = 1.0 / (THETA ** (np.arange(0, DH - 1, 2, dtype=np.float64) / DH))
    ang = np.arange(L, dtype=np.float64)[:, None] * freqs[None, :]  # (L, 64)
    cos = np.cos(ang)
    sin = np.sin(ang)
    cosT = np.empty((P, L), np.float32)
    sinT = np.empty((P, L), np.float32)
    cosT[0::2, :] = cos.T
    cosT[1::2, :] = cos.T
    sinT[0::2, :] = -sin.T
    sinT[1::2, :] = sin.T
    return cosT, sinT


def _host_masks():
    k = np.arange(P)[:, None]
    q = np.arange(256)[None, :]
    import ml_dtypes
    m = np.stack([
        (k <= q), (k <= q - 128), (k >= q + 1), (k >= q - 127),
    ]).astype(ml_dtypes.bfloat16)
    return m.reshape(4 * P, 256)


def _pack_core_inputs(x, Wq, Wk, Wv, Wo, n, g):
    """Prepacked per-core inputs; long contiguous per-partition DMA runs."""
    xT = np.ascontiguousarray(x[n].T)                      # (E, L)
    # xq[qtr*128+p, kt*512+c] = xT[kt*128+p, qtr*512+c]
    xq = xT.reshape(NKT, P, 4, 512).transpose(2, 1, 0, 3).reshape(4 * P, NKT * 512)
    # wqp[h*128+p, kt*128+c] = Wq.T[kt*128+p, g*1024+h*128+c]
    wqT = Wq[g * 1024:(g + 1) * 1024, :].T                 # (E, 1024)
    wqp = wqT.reshape(NKT, P, NH, DH).transpose(2, 1, 0, 3).reshape(NH * P, NKT * DH)
    xq = np.ascontiguousarray(xq)
    wqp = np.ascontiguousarray(wqp)
    # wkv[p, kt*512+j]: j<256 -> Wk.T slice, j>=256 -> Wv.T slice
    wkT = Wk[g * 256:(g + 1) * 256, :].T.reshape(NKT, P, 256)
    wvT = Wv[g * 256:(g + 1) * 256, :].T.reshape(NKT, P, 256)
    wkvp = np.concatenate([wkT, wvT], axis=2)              # (kt, p, 512)
    wkvp = wkvp.transpose(1, 0, 2).reshape(P, NKT * 512)
    woT = Wo[:, g * 1024:(g + 1) * 1024].T                 # (1024, E)
    import ml_dtypes
    return {
        "xq": xq.astype(ml_dtypes.bfloat16),
        "wqp": wqp.astype(ml_dtypes.bfloat16),
        "wkv": np.ascontiguousarray(wkvp).astype(ml_dtypes.bfloat16),
        "woT": np.ascontiguousarray(woT).astype(ml_dtypes.bfloat16),
    }


def kernel(x, Wq, Wk, Wv, Wo):
    global _NC
    x = np.asarray(x, np.float32)
    Wq = np.asarray(Wq, np.float32)
    Wk = np.asarray(Wk, np.float32)
    Wv = np.asarray(Wv, np.float32)
    Wo = np.asarray(Wo, np.float32)

    if _NC is None:
        _NC = build_nc()
    nc = _NC

    cosT, sinT = _host_tables()
    masks = _host_masks()
    in_maps = []
    for c in range(8):
        n, g = c % 4, c // 4
        m = _pack_core_inputs(x, Wq, Wk, Wv, Wo, n, g)
        m.update({"cosT": cosT, "sinT": sinT, "masks": masks})
        in_maps.append(m)

    from concourse.bass_utils import run_bass_kernel_spmd
    res = run_bass_kernel_spmd(nc, in_maps, list(range(8)), trace=False)
    out = np.empty((N, L, E), np.float32)
    for n_ in range(4):
        out[n_] = res.results[n_]["out"] + res.results[4 + n_]["out"]
    return out


if __name__ == "__main__":
    rng = np.random.default_rng(0)
    x = rng.standard_normal((N, L, E), dtype=np.float32)
    Wq = (rng.standard_normal((E, E), dtype=np.float32) * 0.02)
    Wk = (rng.standard_normal((E // D, E), dtype=np.float32) * 0.02)
    Wv = (rng.standard_normal((E // D, E), dtype=np.float32) * 0.02)
    Wo = (rng.standard_normal((E, E), dtype=np.float32) * 0.02)
    print(kernel(x, Wq, Wk, Wv, Wo).shape)

